# revision 24
# baseline (speedup 1.0000x reference)
"""Trainium2 Bass kernel for nn_AugmentPipe (gated flips / 90-degree rots /
reflect-pad integer translation), data-parallel over the batch on 8 cores.

The whole pipeline is a per-sample separable gather:
    out[y, x, c] = in[a[y], b[x], c]            (no transpose), or
    out[y, x, c] = in[a[x], b[y], c]            (rot 90/270)
where a, b are per-sample reflect-translate index vectors (each one +-1
"main" run >= 224 long plus at most one +-1 "edge" run <= 32 at the head or
tail) and the transpose flag comes from rot_w.

v4 load path: per image the host picks
  rev = (main run descends)     -> use a[::-1]      (makes the main run ASC)
  s   = 224 if edge at tail     -> use roll(a'', s) (moves the edge to 0..31)
so the device loads ONLY positions 32..255 -- always a single ascending +-1
row run. Positions 0..31 (which may contain the reflection edge) are never
touched on device: the host computes those 32 gathered rows with numpy and
patches them into the output (rows 0..31 of plain images, cols 0..31 of
transposed ones); a tiny gpsimd memset zeroes the unloaded partitions so
the PE transpose never sees NaN garbage (0*NaN would contaminate columns).

Position k lives at partition k//2, h-block k%2, so each partition's two
rows are DRAM-consecutive and the whole load is ONE dma_start with a
runtime-register offset producing 112 6KB descriptors (HWDGE descriptor
generation at ~12ns/desc is the per-ring throughput cap, so descriptor
count is the currency). The PE transpose then emits y-interleaved columns
(y = 2c + hk); the PSUM eviction un-interleaves with a stride-6
destination AP, paired into 2 copies/image via 2-bank PSUM tiles. Loads
alternate the two HWDGE rings by image parity, the untransposed store
issues on sync and the transposed one on scalar, balancing both rings'
descriptor load. The device result is the row-rolled/row-reversed
augmented image; the host un-rolls/un-reverses with numpy after gathering.

Column gather by b runs on DVE as reg-offset padded copies, a PE fp32
transpose always runs, and 2 cond-predicated stores pick the orientation.
Loads issue on the sync HWDGE ring; stores + PSUM evictions on the scalar
(ACT) ring.
"""
import sys

for _p in ("/opt/trn_rl_repo",):
    if _p not in sys.path:
        sys.path.insert(0, _p)

import numpy as np

N_CORES = 8
N, H, W, C = 128, 256, 256, 3
PER_CORE = N // N_CORES
ROW_ELEMS = W * C  # 768
PAD = 96  # 32 pixels of slack around each data block (elements)

# M1 (row-loaded) free-dim layout, in elements:
#   [96 lead pad][768 h0][768 h1][96 tail pad]  -> width 1728
M1_LEAD = PAD
M1_HSTRIDE = ROW_ELEMS
M1_W = PAD + 2 * ROW_ELEMS + PAD

# N (column-gathered) free-dim layout: [96 lead][768 h0][96 shared pad]
# [768 h1][96 tail][edge dump]. The dump must cover BOTH h-windows of the
# 2-block edge copy (stride 864) plus the 96-elem window itself -> 3456.
N_LEAD = PAD
N_HSTRIDE = ROW_ELEMS + PAD  # 864
N_DUMP = PAD + 2 * ROW_ELEMS + PAD + ROW_ELEMS + PAD  # edge dump start: 2496
N_W = N_DUMP + N_HSTRIDE + PAD  # 2496 + 864 + 96 = 3456

EDGE_PIX = 32
HEAD_ROWS = 32  # rows 0..31 come from the per-image dma_gather

# per-image int32 params: sync block (2), act block (2), dve block (5)
NPARAM = 9
SYNC_BASE = 0                      # [m_off if i even else 0, c_nt]
ACT_BASE = 2 * PER_CORE            # [m_off if i odd else 0, c_tr]
DVE_BASE = 4 * PER_CORE            # [m_src, m_dst, e_src, e_dst, R]
RBATCH = 4                         # images per reg_load batch


def _derive_maps(xflip_w, xflip_gate, yflip_w, yflip_gate, rot_w, rot_gate,
                 trans_w, trans_gate):
    """Replicate the reference gate logic; return (a[N,256], b[N,256], tr[N])."""
    f32 = np.float32
    n = xflip_w.shape[0]
    wx = np.where(np.asarray(xflip_gate).reshape(n) < f32(1.0),
                  np.asarray(xflip_w).reshape(n), 0)
    wy = np.where(np.asarray(yflip_gate).reshape(n) < f32(1.0),
                  np.asarray(yflip_w).reshape(n), 0)
    rw = np.where(np.asarray(rot_gate).reshape(n) < f32(1.0),
                  np.asarray(rot_w).reshape(n), 0)
    tw = np.asarray(trans_w, dtype=np.float32).reshape(2, n) * f32(2.0) - f32(1.0)
    tg = np.asarray(trans_gate).reshape(n)
    tw = np.where(tg[None, :] < f32(1.0), tw, f32(0.0)).astype(np.float32)
    tx = np.round((tw[0] * f32(W)) * f32(0.125)).astype(np.int32)
    ty = np.round((tw[1] * f32(H)) * f32(0.125)).astype(np.int32)

    idx = np.arange(W)
    xi = (W - 1) - np.abs((W - 1) - (idx[None, :] - tx[:, None]) % (2 * W - 2))
    yi = (H - 1) - np.abs((H - 1) - (idx[None, :] + ty[:, None]) % (2 * H - 2))

    xftot = (wx == 1) ^ ((rw == 1) | (rw == 2))
    yftot = (wy == 1) ^ ((rw == 2) | (rw == 3))
    tr = (rw == 1) | (rw == 3)

    a = np.where(tr[:, None], xi, yi)
    a = np.where(yftot[:, None], (H - 1) - a, a)
    b = np.where(tr[:, None], yi, xi)
    b = np.where(xftot[:, None], (W - 1) - b, b)
    return a.astype(np.int64), b.astype(np.int64), tr


def _fit_template(b):
    """Fit b (one +-1 main run >=224 plus <=1 edge run <=32) to the fixed
    4-copy DVE template; return the int32 element offsets + direction flag
    [m_src, m_dst, e_src, e_dst, R]."""
    d = np.diff(b)
    assert np.all(np.abs(d) == 1), b
    change = np.nonzero(d[1:] != d[:-1])[0]
    assert len(change) <= 1, b
    if len(change) == 0:
        runs = [(0, W, int(d[0]))]
    else:
        # the pivot position can belong to either run; pick the split whose
        # short run is <= EDGE_PIX
        c0 = int(change[0])
        runs = None
        for cut in (c0 + 1, c0 + 2):
            r = [(0, cut, int(d[0])), (cut, W, int(d[cut]))]
            lens = sorted(e - s for s, e, _ in r)
            if lens[0] <= EDGE_PIX and lens[1] >= W - EDGE_PIX:
                runs = r
                break
        assert runs is not None, (b, c0)
    if len(runs) == 1:
        main, edge = runs[0], None
    else:
        r0, r1 = runs
        if (r0[1] - r0[0]) >= (r1[1] - r1[0]):
            main, edge = r0, r1
        else:
            main, edge = r1, r0
    mp, mq, md = main
    assert mq - mp >= W - EDGE_PIX, (b, runs)

    # main direction decides the branch: R=0 -> asc main + desc edge,
    # R=1 -> desc main + asc edge
    R = 0 if md == 1 else 1
    m_src = M1_LEAD + 3 * int(b[mp])
    m_dst = N_LEAD + 3 * mp

    if edge is not None:
        ep, eq, ed = edge
        assert eq - ep <= EDGE_PIX and ed == -md, (b, runs)
        if ep == 0:
            wstart = eq - EDGE_PIX  # head edge: window [eq-32, eq)
        else:
            assert eq == W, (b, runs)
            wstart = ep             # tail edge: window [ep, ep+32)
        v0 = int(b[ep]) + ed * (wstart - ep)  # value at window start
        e_src = M1_LEAD + 3 * v0
        e_dst = N_LEAD + 3 * wstart
        assert e_src >= 0 and e_dst >= 0, (b, runs, e_src, e_dst)
    else:
        # taken branch's edge copy still runs; point it at the dump
        e_src = M1_LEAD if md == -1 else M1_LEAD + 3 * (EDGE_PIX - 1)
        e_dst = N_DUMP

    return [m_src, m_dst, e_src, e_dst, R]


def _row_plan(a):
    """Row-map a -> (rev, s, idx32, h0row, h1row).

    rev: use a[::-1] so the main run ascends.
    s: roll amount (0 or 224) moving the edge run into positions 0..31.
    idx32: literal source rows for device positions 0..31.
    h0row/h1row: source rows for device positions 32 / 128 (ascending run).
    """
    d = np.diff(a)
    assert np.all(np.abs(d) == 1), a
    # main-run direction = direction of the longer run
    change = np.nonzero(d[1:] != d[:-1])[0]
    assert len(change) <= 1, a
    if len(change) == 0:
        asc = int(d[0]) == 1
    else:
        c0 = int(change[0])
        head_len = c0 + 1
        asc = (int(d[-1]) == 1) if head_len <= EDGE_PIX else (int(d[0]) == 1)
    rev = not asc
    aa = a[::-1] if rev else a
    dd = np.diff(aa)
    change2 = np.nonzero(dd[1:] != dd[:-1])[0]
    s = 0
    if len(change2) == 1 and int(change2[0]) + 1 >= W // 2:
        s = W - EDGE_PIX  # edge at the tail -> roll it to positions 0..31
    a3 = np.roll(aa, -s) if s else aa
    # positions >= 32 must now be a single ascending run
    d3 = np.diff(a3[HEAD_ROWS:])
    assert np.all(d3 == 1), (a, rev, s)
    return rev, s, a3[:HEAD_ROWS].copy(), int(a3[HEAD_ROWS]), int(a3[128])


_NC_CACHE = {}


def _build_module():
    key = "nc"
    if key in _NC_CACHE:
        return _NC_CACHE[key]
    import concourse.bacc as bacc
    import concourse.bass as bass
    import concourse.mybir as mybir
    import concourse.tile as tile
    from concourse.ap import AP

    DT = mybir.dt.bfloat16  # rel-err gate is 2e-2; bf16 adds ~4e-3 and
    PDT = mybir.dt.float32  # halves HBM traffic + engine element counts
    nc = bacc.Bacc(None, num_swdge_queues=2)
    images = nc.dram_tensor("images", [PER_CORE, H, W, C], DT, kind="ExternalInput")
    identity_in = nc.dram_tensor("identity_in", [128, 128], DT, kind="ExternalInput")
    params = nc.dram_tensor("params", [1, NPARAM * PER_CORE], mybir.dt.int32,
                            kind="ExternalInput")
    out = nc.dram_tensor("out", [PER_CORE, H, W, C], DT, kind="ExternalOutput")

    img_elems = H * W * C

    with tile.TileContext(nc) as tc:
        with (
            tc.tile_pool(name="const", bufs=1) as const_pool,
            tc.tile_pool(name="m1", bufs=5) as m1_pool,
            tc.tile_pool(name="ncg", bufs=4) as n_pool,
            tc.tile_pool(name="tt", bufs=4) as t_pool,
            tc.tile_pool(name="psum", bufs=4, space="PSUM") as psum_pool,
        ):
            par_t = const_pool.tile([1, NPARAM * PER_CORE], mybir.dt.int32)
            nc.sync.dma_start(par_t[:], params[:])
            ident = const_pool.tile([128, 128], DT)
            nc.sync.dma_start(ident[:], identity_in[:])

            dve = nc.vector.engine
            act = nc.scalar.engine
            sp = nc.sync.engine

            img_t = images[:].tensor
            out_t = out[:].tensor

            for i in range(PER_CORE):
                # --- 1. row load: rows of a3 -> M1 (position k at
                # partition k//2, h-block k%2) ---
                m1 = m1_pool.tile([128, M1_W], DT, tag="m1")
                m1t = m1[:].tensor

                # 1a. positions 0..31 are host-patched; zero them so the PE
                # transpose never sums NaN garbage (0*NaN = NaN)
                nc.gpsimd.memset(m1[0:16, M1_LEAD:M1_LEAD + 2 * ROW_ELEMS], 0.0)

                if i % RBATCH == 0:
                    nb = min(RBATCH, PER_CORE - i)
                    sregs = [nc.alloc_register(sp, f"ld{i}_{j}")
                             for j in range(2 * nb)]
                    nc.sync.reg_load(
                        sregs, par_t[0:1, SYNC_BASE + 2 * i:
                                     SYNC_BASE + 2 * (i + nb)])
                    aregs = [nc.alloc_register(act, f"st{i}_{j}")
                             for j in range(2 * nb)]
                    nc.scalar.reg_load(
                        aregs, par_t[0:1, ACT_BASE + 2 * i:
                                     ACT_BASE + 2 * (i + nb)])
                sr = sregs[2 * (i % RBATCH):2 * (i % RBATCH) + 2]
                ar = aregs[2 * (i % RBATCH):2 * (i % RBATCH) + 2]

                # 1b. positions 32..255: single ascending run = 112 paired
                # rows, ONE dma_start; ring alternates with image parity
                ld_dst = m1[16:128, M1_LEAD:M1_LEAD + 2 * ROW_ELEMS]
                ld_dims = [[2 * ROW_ELEMS, 112], [1, 2 * ROW_ELEMS]]
                if i % 2 == 0:
                    nc.sync.dma_start(
                        ld_dst, AP(img_t, bass.RuntimeValue(sr[0]), ld_dims))
                else:
                    nc.scalar.dma_start(
                        ld_dst, AP(img_t, bass.RuntimeValue(ar[0]), ld_dims))

                # --- 2. column gather by b: M1 -> Ntile (4 reg-offset copies) ---
                ntile = n_pool.tile([128, N_W], DT, tag="ncg")
                ntt = ntile[:].tensor
                p_m1 = [M1_W, 128]
                p_n = [N_W, 128]
                # per-image virtual registers; 5 per image (main src/dst,
                # edge src/dst, R flag), loaded per image pair. The R flag
                # branches ONLY the DVE stream: R=0 runs {asc main, desc
                # edge}, R=1 runs {desc main, asc edge} - halving DVE work
                # vs executing all four direction variants.
                if i % RBATCH == 0:
                    nb = min(RBATCH, PER_CORE - i)
                    pair_regs = [nc.alloc_register(dve, f"cg{i}_{j}")
                                 for j in range(5 * nb)]
                    nc.vector.reg_load(
                        pair_regs, par_t[0:1, DVE_BASE + 5 * i:
                                         DVE_BASE + 5 * (i + nb)])
                dve_regs = pair_regs[5 * (i % RBATCH):5 * (i % RBATCH) + 5]
                with tc.If(bass.RuntimeValue(dve_regs[4]) < 1) as cmp:
                    nc.vector.tensor_copy(
                        AP(ntt, dve_regs[1], [p_n, [N_HSTRIDE, 2], [1, ROW_ELEMS]]),
                        AP(m1t, dve_regs[0], [p_m1, [M1_HSTRIDE, 2], [1, ROW_ELEMS]]))
                    nc.vector.tensor_copy(
                        AP(ntt, dve_regs[3], [p_n, [N_HSTRIDE, 2], [1, 3 * EDGE_PIX]]),
                        AP(m1t, dve_regs[2], [p_m1, [M1_HSTRIDE, 2], [-3, EDGE_PIX], [1, C]]))
                with cmp.Else():
                    nc.vector.tensor_copy(
                        AP(ntt, dve_regs[1], [p_n, [N_HSTRIDE, 2], [1, ROW_ELEMS]]),
                        AP(m1t, dve_regs[0], [p_m1, [M1_HSTRIDE, 2], [-3, W], [1, C]]))
                    nc.vector.tensor_copy(
                        AP(ntt, dve_regs[3], [p_n, [N_HSTRIDE, 2], [1, 3 * EDGE_PIX]]),
                        AP(m1t, dve_regs[2], [p_m1, [M1_HSTRIDE, 2], [1, 3 * EDGE_PIX]]))

                # --- 3. pixel transpose Ntile -> Ttile via PE (exact fp32) ---
                # Ntile partition c', block hk holds G row y = 2c'+hk, so
                # the transpose emits y-interleaved columns; the eviction
                # un-interleaves with a stride-6 dst AP, pairing both hu
                # quadrants through one 2-bank PSUM tile (1 copy per hk)
                ttile = t_pool.tile([128, 2, ROW_ELEMS], DT, tag="tt")
                ttt = ttile[:].tensor
                tt_p = [2 * ROW_ELEMS, 128]
                for hk in range(2):
                    # each (hu, c) gets its own aligned 128-col PSUM block;
                    # the eviction re-interleaves channel and y in one pass
                    pt = psum_pool.tile([128, 1024], DT, tag="pt")
                    ptt = pt[:].tensor
                    for hu in range(2):
                        for c in range(C):
                            stat = AP(ntt, N_LEAD + hk * N_HSTRIDE + 3 * (hu * 128) + c,
                                      [p_n, [3, 128]])
                            nc.tensor.transpose(
                                AP(ptt, 384 * hu + 128 * c, [[1024, 128], [1, 128]]),
                                stat, ident[:])
                    nc.vector.tensor_copy(
                        AP(ttt, 3 * hk, [tt_p, [ROW_ELEMS, 2], [1, C], [6, 128]]),
                        AP(ptt, 0, [[1024, 128], [384, 2], [128, C], [1, 128]]))

                # --- 4. predicated stores: untransposed (sync ring) or
                # transposed (scalar ring) ---
                n_src = AP(ntt, N_LEAD, [p_n, [N_HSTRIDE, 2], [1, ROW_ELEMS]])
                cn = nc.sync.snap(sr[1], min_val=0, max_val=1)
                ct = nc.scalar.snap(ar[1], min_val=0, max_val=1)
                dram_n = AP(out_t, i * img_elems,
                            [[2 * ROW_ELEMS, 128], [ROW_ELEMS, 2], [1, ROW_ELEMS]])
                nc.sync.dma_start(dram_n, n_src, cond=cn)
                dram_t = AP(out_t, i * img_elems,
                            [[ROW_ELEMS, 128], [128 * ROW_ELEMS, 2], [1, ROW_ELEMS]])
                nc.scalar.dma_start(dram_t, ttile[:], cond=ct)

    nc.finalize()
    _NC_CACHE[key] = nc
    return nc


def _make_in_maps(images, a, b, tr):
    import ml_dtypes

    ident = np.eye(128, dtype=ml_dtypes.bfloat16)
    images = images.astype(ml_dtypes.bfloat16)
    in_maps = []
    for core in range(N_CORES):
        s0 = core * PER_CORE
        par = np.zeros((1, NPARAM * PER_CORE), np.int32)
        for i in range(PER_CORE):
            rev, s, idx32, h0row, h1row = _row_plan(a[s0 + i])
            m_off = (i * H + h0row) * ROW_ELEMS
            t = bool(tr[s0 + i])
            par[0, SYNC_BASE + 2 * i + 0] = m_off if i % 2 == 0 else 0
            par[0, SYNC_BASE + 2 * i + 1] = 0 if t else 1
            par[0, ACT_BASE + 2 * i + 0] = m_off if i % 2 == 1 else 0
            par[0, ACT_BASE + 2 * i + 1] = 1 if t else 0
            par[0, DVE_BASE + 5 * i:DVE_BASE + 5 * i + 5] = \
                _fit_template(b[s0 + i])
        in_maps.append({
            "images": images[s0:s0 + PER_CORE],
            "identity_in": ident,
            "params": par,
        })
    return in_maps


def _postprocess(raw, images, a, b, tr):
    """Patch the 32 host-handled edge rows into the device result, then undo
    the per-image row roll/reversal the device computed with.

    Plain images come back as S = roll(flip?(G), -s) on axis 0 with rows
    0..31 garbage; transposed ones as S^T with cols 0..31 garbage.
    """
    out = raw
    for n in range(raw.shape[0]):
        rev, s, idx32, _, _ = _row_plan(a[n])
        head = images[n][idx32][:, b[n]]  # [32, W, C] device positions 0..31
        img = raw[n]
        if tr[n]:
            img[:, :HEAD_ROWS] = head.transpose(1, 0, 2)
        else:
            img[:HEAD_ROWS] = head
        if not rev and s == 0:
            continue
        ax = 1 if tr[n] else 0
        if s:
            img = np.roll(img, s, axis=ax)
        if rev:
            img = np.flip(img, axis=ax)
        out[n] = img
    return out


def kernel(images, xflip_w, xflip_gate, yflip_w, yflip_gate, rot_w, rot_gate,
           trans_w, trans_gate):
    from concourse.bass_utils import run_bass_kernel_spmd

    images = np.ascontiguousarray(np.asarray(images, dtype=np.float32))
    a, b, tr = _derive_maps(xflip_w, xflip_gate, yflip_w, yflip_gate,
                            rot_w, rot_gate, trans_w, trans_gate)
    nc = _build_module()
    in_maps = _make_in_maps(images, a, b, tr)
    res = run_bass_kernel_spmd(nc, in_maps, list(range(N_CORES))).results
    raw = np.concatenate([np.asarray(res[c]["out"], dtype=np.float32)
                          for c in range(N_CORES)], axis=0)
    return _postprocess(raw, images, a, b, tr)


# revision 29
# speedup vs baseline: 1.6119x; 1.6119x over previous
"""Trainium2 Bass kernel for nn_AugmentPipe (gated flips / 90-degree rots /
reflect-pad integer translation), data-parallel over the batch on 8 cores.

The whole pipeline is a per-sample separable gather:
    out[y, x, c] = in[a[y], b[x], c]            (no transpose), or
    out[y, x, c] = in[a[x], b[y], c]            (rot 90/270)
where a, b are per-sample reflect-translate index vectors (each one +-1
"main" run >= 224 long plus at most one +-1 "edge" run <= 32 at the head or
tail) and the transpose flag comes from rot_w.

v4 load path: per image the host picks
  rev = (main run descends)     -> use a[::-1]      (makes the main run ASC)
  s   = 224 if edge at tail     -> use roll(a'', s) (moves the edge to 0..31)
so the device loads ONLY positions 32..255 -- always a single ascending +-1
row run. Positions 0..31 (which may contain the reflection edge) are never
touched on device: the host computes those 32 gathered rows with numpy and
patches them into the output (rows 0..31 of plain images, cols 0..31 of
transposed ones); a tiny gpsimd memset zeroes the unloaded partitions so
the PE transpose never sees NaN garbage (0*NaN would contaminate columns).

Position k lives at partition k//2, h-block k%2, so each partition's two
rows are DRAM-consecutive and the whole load is ONE dma_start with a
runtime-register offset producing 112 6KB descriptors (HWDGE descriptor
generation at ~12ns/desc is the per-ring throughput cap, so descriptor
count is the currency). The PE transpose then emits y-interleaved columns
(y = 2c + hk); the PSUM eviction un-interleaves with a stride-6
destination AP, paired into 2 copies/image via 2-bank PSUM tiles. Loads
alternate the two HWDGE rings by image parity, the untransposed store
issues on sync and the transposed one on scalar, balancing both rings'
descriptor load. The device result is the row-rolled/row-reversed
augmented image; the host un-rolls/un-reverses with numpy after gathering.

Column gather by b runs on DVE as reg-offset padded copies, a PE fp32
transpose always runs, and 2 cond-predicated stores pick the orientation.
Loads issue on the sync HWDGE ring; stores + PSUM evictions on the scalar
(ACT) ring.
"""
import sys

for _p in ("/opt/trn_rl_repo",):
    if _p not in sys.path:
        sys.path.insert(0, _p)

import numpy as np

N_CORES = 8
N, H, W, C = 128, 256, 256, 3
PER_CORE = N // N_CORES
ROW_ELEMS = W * C  # 768
PAD = 96  # 32 pixels of slack around each data block (elements)

# M1 (row-loaded) free-dim layout, in elements:
#   [96 lead pad][768 h0][768 h1][96 tail pad]  -> width 1728
M1_LEAD = PAD
M1_HSTRIDE = ROW_ELEMS
M1_W = PAD + 2 * ROW_ELEMS + PAD

# N (column-gathered) free-dim layout: [96 lead][768 h0][96 shared pad]
# [768 h1][96 tail][edge dump]. The dump must cover BOTH h-windows of the
# 2-block edge copy (stride 864) plus the 96-elem window itself -> 3456.
N_LEAD = PAD
N_HSTRIDE = ROW_ELEMS + PAD  # 864
N_DUMP = PAD + 2 * ROW_ELEMS + PAD + ROW_ELEMS + PAD  # edge dump start: 2496
N_W = N_DUMP + N_HSTRIDE + PAD  # 2496 + 864 + 96 = 3456

EDGE_PIX = 32
HEAD_ROWS = 32  # rows 0..31 come from the per-image dma_gather

# per-image int32 params: sync block (3), act block (3), dve block (5)
# loads issue on the parity engine, stores on the opposite one
NPARAM = 11
SYNC_BASE = 0                      # [m_off (even), c_nt (odd), c_tr (odd)]
ACT_BASE = 3 * PER_CORE            # [m_off (odd), c_nt (even), c_tr (even)]
DVE_BASE = 6 * PER_CORE            # [m_src, m_dst, e_src, e_dst, R]
RBATCH = 4                         # images per reg_load batch


def _derive_maps(xflip_w, xflip_gate, yflip_w, yflip_gate, rot_w, rot_gate,
                 trans_w, trans_gate):
    """Replicate the reference gate logic; return (a[N,256], b[N,256], tr[N])."""
    f32 = np.float32
    n = xflip_w.shape[0]
    wx = np.where(np.asarray(xflip_gate).reshape(n) < f32(1.0),
                  np.asarray(xflip_w).reshape(n), 0)
    wy = np.where(np.asarray(yflip_gate).reshape(n) < f32(1.0),
                  np.asarray(yflip_w).reshape(n), 0)
    rw = np.where(np.asarray(rot_gate).reshape(n) < f32(1.0),
                  np.asarray(rot_w).reshape(n), 0)
    tw = np.asarray(trans_w, dtype=np.float32).reshape(2, n) * f32(2.0) - f32(1.0)
    tg = np.asarray(trans_gate).reshape(n)
    tw = np.where(tg[None, :] < f32(1.0), tw, f32(0.0)).astype(np.float32)
    tx = np.round((tw[0] * f32(W)) * f32(0.125)).astype(np.int32)
    ty = np.round((tw[1] * f32(H)) * f32(0.125)).astype(np.int32)

    idx = np.arange(W)
    xi = (W - 1) - np.abs((W - 1) - (idx[None, :] - tx[:, None]) % (2 * W - 2))
    yi = (H - 1) - np.abs((H - 1) - (idx[None, :] + ty[:, None]) % (2 * H - 2))

    xftot = (wx == 1) ^ ((rw == 1) | (rw == 2))
    yftot = (wy == 1) ^ ((rw == 2) | (rw == 3))
    tr = (rw == 1) | (rw == 3)

    a = np.where(tr[:, None], xi, yi)
    a = np.where(yftot[:, None], (H - 1) - a, a)
    b = np.where(tr[:, None], yi, xi)
    b = np.where(xftot[:, None], (W - 1) - b, b)
    return a.astype(np.int64), b.astype(np.int64), tr


def _fit_template(b):
    """Fit b (one +-1 main run >=224 plus <=1 edge run <=32) to the fixed
    4-copy DVE template; return the int32 element offsets + direction flag
    [m_src, m_dst, e_src, e_dst, R]."""
    d = np.diff(b)
    assert np.all(np.abs(d) == 1), b
    change = np.nonzero(d[1:] != d[:-1])[0]
    assert len(change) <= 1, b
    if len(change) == 0:
        runs = [(0, W, int(d[0]))]
    else:
        # the pivot position can belong to either run; pick the split whose
        # short run is <= EDGE_PIX
        c0 = int(change[0])
        runs = None
        for cut in (c0 + 1, c0 + 2):
            r = [(0, cut, int(d[0])), (cut, W, int(d[cut]))]
            lens = sorted(e - s for s, e, _ in r)
            if lens[0] <= EDGE_PIX and lens[1] >= W - EDGE_PIX:
                runs = r
                break
        assert runs is not None, (b, c0)
    if len(runs) == 1:
        main, edge = runs[0], None
    else:
        r0, r1 = runs
        if (r0[1] - r0[0]) >= (r1[1] - r1[0]):
            main, edge = r0, r1
        else:
            main, edge = r1, r0
    mp, mq, md = main
    assert mq - mp >= W - EDGE_PIX, (b, runs)

    # main direction decides the branch: R=0 -> asc main + desc edge,
    # R=1 -> desc main + asc edge
    R = 0 if md == 1 else 1
    m_src = M1_LEAD + 3 * int(b[mp])
    m_dst = N_LEAD + 3 * mp

    if edge is not None:
        ep, eq, ed = edge
        assert eq - ep <= EDGE_PIX and ed == -md, (b, runs)
        if ep == 0:
            wstart = eq - EDGE_PIX  # head edge: window [eq-32, eq)
        else:
            assert eq == W, (b, runs)
            wstart = ep             # tail edge: window [ep, ep+32)
        v0 = int(b[ep]) + ed * (wstart - ep)  # value at window start
        e_src = M1_LEAD + 3 * v0
        e_dst = N_LEAD + 3 * wstart
        assert e_src >= 0 and e_dst >= 0, (b, runs, e_src, e_dst)
    else:
        # taken branch's edge copy still runs; point it at the dump
        e_src = M1_LEAD if md == -1 else M1_LEAD + 3 * (EDGE_PIX - 1)
        e_dst = N_DUMP

    return [m_src, m_dst, e_src, e_dst, R]


def _row_plan(a):
    """Row-map a -> (rev, s, idx32, h0row, h1row).

    rev: use a[::-1] so the main run ascends.
    s: roll amount (0 or 224) moving the edge run into positions 0..31.
    idx32: literal source rows for device positions 0..31.
    h0row/h1row: source rows for device positions 32 / 128 (ascending run).
    """
    d = np.diff(a)
    assert np.all(np.abs(d) == 1), a
    # main-run direction = direction of the longer run
    change = np.nonzero(d[1:] != d[:-1])[0]
    assert len(change) <= 1, a
    if len(change) == 0:
        asc = int(d[0]) == 1
    else:
        c0 = int(change[0])
        head_len = c0 + 1
        asc = (int(d[-1]) == 1) if head_len <= EDGE_PIX else (int(d[0]) == 1)
    rev = not asc
    aa = a[::-1] if rev else a
    dd = np.diff(aa)
    change2 = np.nonzero(dd[1:] != dd[:-1])[0]
    s = 0
    if len(change2) == 1 and int(change2[0]) + 1 >= W // 2:
        s = W - EDGE_PIX  # edge at the tail -> roll it to positions 0..31
    a3 = np.roll(aa, -s) if s else aa
    # positions >= 32 must now be a single ascending run
    d3 = np.diff(a3[HEAD_ROWS:])
    assert np.all(d3 == 1), (a, rev, s)
    return rev, s, a3[:HEAD_ROWS].copy(), int(a3[HEAD_ROWS]), int(a3[128])


_NC_CACHE = {}


def _build_module():
    key = "nc"
    if key in _NC_CACHE:
        return _NC_CACHE[key]
    import concourse.bacc as bacc
    import concourse.bass as bass
    import concourse.mybir as mybir
    import concourse.tile as tile
    from concourse.ap import AP

    DT = mybir.dt.bfloat16  # rel-err gate is 2e-2; bf16 adds ~4e-3 and
    PDT = mybir.dt.float32  # halves HBM traffic + engine element counts
    nc = bacc.Bacc(None, num_swdge_queues=2)
    images = nc.dram_tensor("images", [PER_CORE, H, W, C], DT, kind="ExternalInput")
    identity_in = nc.dram_tensor("identity_in", [128, 128], DT, kind="ExternalInput")
    params = nc.dram_tensor("params", [1, NPARAM * PER_CORE], mybir.dt.int32,
                            kind="ExternalInput")
    out = nc.dram_tensor("out", [PER_CORE, H, W, C], DT, kind="ExternalOutput")

    img_elems = H * W * C

    with tile.TileContext(nc) as tc:
        with (
            tc.tile_pool(name="const", bufs=1) as const_pool,
            tc.tile_pool(name="m1", bufs=5) as m1_pool,
            tc.tile_pool(name="ncg", bufs=4) as n_pool,
            tc.tile_pool(name="tt", bufs=4) as t_pool,
            tc.tile_pool(name="psum", bufs=4, space="PSUM") as psum_pool,
        ):
            par_t = const_pool.tile([1, NPARAM * PER_CORE], mybir.dt.int32)
            nc.sync.dma_start(par_t[:], params[:])
            ident = const_pool.tile([128, 128], DT)
            nc.sync.dma_start(ident[:], identity_in[:])

            dve = nc.vector.engine
            act = nc.scalar.engine
            sp = nc.sync.engine

            img_t = images[:].tensor
            out_t = out[:].tensor

            for i in range(PER_CORE):
                # --- 1. row load: rows of a3 -> M1 (position k at
                # partition k//2, h-block k%2) ---
                m1 = m1_pool.tile([128, M1_W], DT, tag="m1")
                m1t = m1[:].tensor

                # 1a. positions 0..31 are host-patched; zero them so the PE
                # transpose never sums NaN garbage (0*NaN = NaN)
                nc.gpsimd.memset(m1[0:16, M1_LEAD:M1_LEAD + 2 * ROW_ELEMS], 0.0)

                if i % RBATCH == 0:
                    nb = min(RBATCH, PER_CORE - i)
                    sregs = [nc.alloc_register(sp, f"ld{i}_{j}")
                             for j in range(3 * nb)]
                    nc.sync.reg_load(
                        sregs, par_t[0:1, SYNC_BASE + 3 * i:
                                     SYNC_BASE + 3 * (i + nb)])
                    aregs = [nc.alloc_register(act, f"st{i}_{j}")
                             for j in range(3 * nb)]
                    nc.scalar.reg_load(
                        aregs, par_t[0:1, ACT_BASE + 3 * i:
                                     ACT_BASE + 3 * (i + nb)])
                sr = sregs[3 * (i % RBATCH):3 * (i % RBATCH) + 3]
                ar = aregs[3 * (i % RBATCH):3 * (i % RBATCH) + 3]

                # 1b. positions 32..255: single ascending run = 112 paired
                # rows, ONE dma_start; ring alternates with image parity
                ld_dst = m1[16:128, M1_LEAD:M1_LEAD + 2 * ROW_ELEMS]
                ld_dims = [[2 * ROW_ELEMS, 112], [1, 2 * ROW_ELEMS]]
                if i % 2 == 0:
                    nc.sync.dma_start(
                        ld_dst, AP(img_t, bass.RuntimeValue(sr[0]), ld_dims))
                else:
                    nc.scalar.dma_start(
                        ld_dst, AP(img_t, bass.RuntimeValue(ar[0]), ld_dims))

                # --- 2. column gather by b: M1 -> Ntile (4 reg-offset copies) ---
                ntile = n_pool.tile([128, N_W], DT, tag="ncg")
                ntt = ntile[:].tensor
                p_m1 = [M1_W, 128]
                p_n = [N_W, 128]
                # per-image virtual registers; 5 per image (main src/dst,
                # edge src/dst, R flag), loaded per image pair. The R flag
                # branches ONLY the DVE stream: R=0 runs {asc main, desc
                # edge}, R=1 runs {desc main, asc edge} - halving DVE work
                # vs executing all four direction variants.
                if i % RBATCH == 0:
                    nb = min(RBATCH, PER_CORE - i)
                    pair_regs = [nc.alloc_register(dve, f"cg{i}_{j}")
                                 for j in range(5 * nb)]
                    nc.vector.reg_load(
                        pair_regs, par_t[0:1, DVE_BASE + 5 * i:
                                         DVE_BASE + 5 * (i + nb)])
                dve_regs = pair_regs[5 * (i % RBATCH):5 * (i % RBATCH) + 5]
                with tc.If(bass.RuntimeValue(dve_regs[4]) < 1) as cmp:
                    nc.vector.tensor_copy(
                        AP(ntt, dve_regs[1], [p_n, [N_HSTRIDE, 2], [1, ROW_ELEMS]]),
                        AP(m1t, dve_regs[0], [p_m1, [M1_HSTRIDE, 2], [1, ROW_ELEMS]]))
                    nc.vector.tensor_copy(
                        AP(ntt, dve_regs[3], [p_n, [N_HSTRIDE, 2], [1, 3 * EDGE_PIX]]),
                        AP(m1t, dve_regs[2], [p_m1, [M1_HSTRIDE, 2], [-3, EDGE_PIX], [1, C]]))
                with cmp.Else():
                    nc.vector.tensor_copy(
                        AP(ntt, dve_regs[1], [p_n, [N_HSTRIDE, 2], [1, ROW_ELEMS]]),
                        AP(m1t, dve_regs[0], [p_m1, [M1_HSTRIDE, 2], [-3, W], [1, C]]))
                    nc.vector.tensor_copy(
                        AP(ntt, dve_regs[3], [p_n, [N_HSTRIDE, 2], [1, 3 * EDGE_PIX]]),
                        AP(m1t, dve_regs[2], [p_m1, [M1_HSTRIDE, 2], [1, 3 * EDGE_PIX]]))

                # --- 3. pixel transpose Ntile -> Ttile via PE (exact fp32) ---
                # Ntile partition c', block hk holds G row y = 2c'+hk, so
                # the transpose emits y-interleaved columns; the eviction
                # un-interleaves with a stride-6 dst AP, pairing both hu
                # quadrants through one 2-bank PSUM tile (1 copy per hk)
                ttile = t_pool.tile([128, 2, ROW_ELEMS], DT, tag="tt")
                ttt = ttile[:].tensor
                tt_p = [2 * ROW_ELEMS, 128]
                for hk in range(2):
                    # each (hu, c) gets its own aligned 128-col PSUM block;
                    # ttile keeps this blocked order (the HOST reinterprets
                    # transposed images), so evictions are plain contiguous
                    # copies, one on scalar and one on vector
                    pt = psum_pool.tile([128, 1024], DT, tag="pt")
                    ptt = pt[:].tensor
                    for hu in range(2):
                        for c in range(C):
                            stat = AP(ntt, N_LEAD + hk * N_HSTRIDE + 3 * (hu * 128) + c,
                                      [p_n, [3, 128]])
                            nc.tensor.transpose(
                                AP(ptt, 384 * hu + 128 * c, [[1024, 128], [1, 128]]),
                                stat, ident[:])
                    ev_dst = ttile[:, hk, :]
                    ev_src = AP(ptt, 0, [[1024, 128], [1, ROW_ELEMS]])
                    if hk == 0:
                        nc.scalar.copy(ev_dst, ev_src)
                    else:
                        nc.vector.tensor_copy(ev_dst, ev_src)

                # --- 4. predicated stores (one real per image), on the
                # opposite-parity engine from the load ---
                n_src = AP(ntt, N_LEAD, [p_n, [N_HSTRIDE, 2], [1, ROW_ELEMS]])
                dram_n = AP(out_t, i * img_elems,
                            [[2 * ROW_ELEMS, 128], [ROW_ELEMS, 2], [1, ROW_ELEMS]])
                # transposed result goes out as a raw [128, 1536] blob with
                # 6KB descriptors; the host untangles the block order
                dram_t = AP(out_t, i * img_elems,
                            [[2 * ROW_ELEMS, 128], [1, 2 * ROW_ELEMS]])
                t_src = AP(ttt, 0, [tt_p, [1, 2 * ROW_ELEMS]])
                if i % 2 == 0:
                    cn = nc.scalar.snap(ar[1], min_val=0, max_val=1)
                    ct = nc.scalar.snap(ar[2], min_val=0, max_val=1)
                    nc.scalar.dma_start(dram_n, n_src, cond=cn)
                    nc.scalar.dma_start(dram_t, t_src, cond=ct)
                else:
                    cn = nc.sync.snap(sr[1], min_val=0, max_val=1)
                    ct = nc.sync.snap(sr[2], min_val=0, max_val=1)
                    nc.sync.dma_start(dram_n, n_src, cond=cn)
                    nc.sync.dma_start(dram_t, t_src, cond=ct)

    nc.finalize()
    _NC_CACHE[key] = nc
    return nc


def _make_in_maps(images, a, b, tr):
    import ml_dtypes

    ident = np.eye(128, dtype=ml_dtypes.bfloat16)
    images = images.astype(ml_dtypes.bfloat16)
    in_maps = []
    for core in range(N_CORES):
        s0 = core * PER_CORE
        par = np.zeros((1, NPARAM * PER_CORE), np.int32)
        for i in range(PER_CORE):
            rev, s, idx32, h0row, h1row = _row_plan(a[s0 + i])
            m_off = (i * H + h0row) * ROW_ELEMS
            t = bool(tr[s0 + i])
            ld, st = (SYNC_BASE, ACT_BASE) if i % 2 == 0 else (ACT_BASE, SYNC_BASE)
            par[0, ld + 3 * i + 0] = m_off
            par[0, st + 3 * i + 1] = 0 if t else 1
            par[0, st + 3 * i + 2] = 1 if t else 0
            par[0, DVE_BASE + 5 * i:DVE_BASE + 5 * i + 5] = \
                _fit_template(b[s0 + i])
        in_maps.append({
            "images": images[s0:s0 + PER_CORE],
            "identity_in": ident,
            "params": par,
        })
    return in_maps


def _postprocess(raw, images, a, b, tr):
    """Patch the 32 host-handled edge rows into the device result, then undo
    the per-image row roll/reversal the device computed with.

    Plain images come back as S = roll(flip?(G), -s) on axis 0 with rows
    0..31 garbage; transposed ones as S^T with cols 0..31 garbage.
    """
    out = raw
    for n in range(raw.shape[0]):
        rev, s, idx32, _, _ = _row_plan(a[n])
        head = images[n][idx32][:, b[n]]  # [32, W, C] device positions 0..31
        img = raw[n]
        if tr[n]:
            # raw blob [u, hk, hu, c, c'] -> [x = 128*hu+u, y = 2*c'+hk, c]
            img = img.reshape(128, 2, 2, C, 128).transpose(
                2, 0, 4, 1, 3).reshape(H, W, C)
            img[:, :HEAD_ROWS] = head.transpose(1, 0, 2)
        else:
            img[:HEAD_ROWS] = head
        ax = 1 if tr[n] else 0
        if s:
            img = np.roll(img, s, axis=ax)
        if rev:
            img = np.flip(img, axis=ax)
        out[n] = img
    return out


def kernel(images, xflip_w, xflip_gate, yflip_w, yflip_gate, rot_w, rot_gate,
           trans_w, trans_gate):
    from concourse.bass_utils import run_bass_kernel_spmd

    images = np.ascontiguousarray(np.asarray(images, dtype=np.float32))
    a, b, tr = _derive_maps(xflip_w, xflip_gate, yflip_w, yflip_gate,
                            rot_w, rot_gate, trans_w, trans_gate)
    nc = _build_module()
    in_maps = _make_in_maps(images, a, b, tr)
    res = run_bass_kernel_spmd(nc, in_maps, list(range(N_CORES))).results
    raw = np.concatenate([np.asarray(res[c]["out"], dtype=np.float32)
                          for c in range(N_CORES)], axis=0)
    return _postprocess(raw, images, a, b, tr)
